# revision 8
# baseline (speedup 1.0000x reference)
"""Trainium2 Bass kernel for nn_CQLoss (composite loss function).

Strategy: pure data parallel over batch dim (64 batches -> 8 per core), all
large tensors travelling as fp8-e4m3. Every loss term is expanded into global
sums of products computed on the PE as PSUM-accumulated DoubleRow Gram-tile
chains (diag of psum += tile^T @ tile' holds the per-column dot products):

  recon*N  = sum g^2 - 2 sum g.z + sum z^2      (g = mapping-gathered rzs)
  pts*N    = host-weighted (x - y)^2 via DVE diff + ACT square-accumulate
  kld*N*V  = sum qV * ln(qV + 2^-9)  (PE: qV (x) ln-tile diag)
  best*N   = subtract/square (tiny, f32)

Host pre-scaling folds every Gram coefficient into C_ZZ (gather rows carry
-2*rz; qv carries 0.4*qy*V with the ln recovering via scale=2.5) so all 144
z/gz/ql tiles accumulate into just THREE psum regions (mainA, gg, mainB) --
three masked-diag reductions total, exactly one of which (mainB, 258 ns)
sits in the end-of-kernel tail.

DMA architecture (the kernel is DMA-bound; the cost model serializes all
transfers on one 360 B/ns device, so wall-clock ~= first-transfer latency +
total transfer time + last-transfer dependent tail):
  - mapping-gathered rows ride in TWO batched SWDGE dma_gather ops (4 batches
    each, 512 rows x 2304B), prepared on the Pool engine from i16 idx tables
    uploaded in cpack and fired by trigger_dma -- triggered transfers skip
    the HWDGE-gen and DGE-delay pipeline stages.
  - direct loads (qv, cpack, zs chunks, ptsgt) are SP-issued HWDGE copies:
    qv first (its 1.46 us transfer hides the later gens' pipeline), gathers
    mid-stream, then ptsgt and the remaining zs batches; batch 7's zs is
    split into four 512B column-slices (512B is the smallest descriptor that
    still runs at full bus rate) so the last transfer gates only 4 matmuls.
  - the scalar output leaves via a kv_writeback prepared mid-kernel (after
    gather A drains the SWDGE ring) and triggered right after the last
    accumulation, collapsing the output pipeline to trigger+transfer+sem;
    no engine waits on the output semaphore -- the runtime's own completion
    barrier covers it.
The remaining tail is structural: 900 ns DMA-sem propagation on the last
input, ~160 ns PE, ~440 ns DVE diag (incl dispatch), ~220 ns Pool trigger
hops, and 900 ns output-sem propagation.

PSUM: mainA/gg/mainB each own a 128-col region in its own bank (the DVE
must not read a psum bank the PE still writes; matmul accumulation chains
must stay contiguous in the PE stream -- tile order WITHIN a chain is free,
which is what lets one region span many waits). Each diag reduction is one
scalar_tensor_tensor (psum * coef * identity, accumulated into one acc
column); the identity tile is built on-chip from a Pool iota (p - f) and a
DVE is_equal. The host sums partitions/cores in float64.
"""

import os
import sys

import numpy as np

for _p in ("/opt/trn_rl_repo", "/root/.axon_site/_ro/trn_rl_repo"):
    if os.path.isdir(_p) and _p not in sys.path:
        sys.path.insert(0, _p)

B, S, D, P, C, V = 64, 128, 2048, 118, 2, 512
PC = P * C  # 236
PCP = 256  # padded pts width
K = D + PCP  # gather row bytes: 2304
N_CORES = 8
BL = B // N_CORES  # 8 batches per core
ALPHA, BETA, GAMMA, EPS = 10.0, 0.1, 1.0, 1e-20
MARKS = (0, 29, 88, 117)
W_MARK = ALPHA * PC / (len(MARKS) * C)  # 295.0
LN_B0 = 2.0 ** -9

# final linear-combination coefficients (applied via the psum diag masks)
C_ZZ = GAMMA / (B * S * D)
C_GZ = -2.0 * GAMMA / (B * S * D)
C_QL = BETA / (B * S * V)

NDT = D // 256  # 8 DoubleRow supertiles per batch

# psum regions: (name, bank-ordered col offset, coefficient)
# banks (512 cols) grouped by chain completion; diag of a region only runs
# after every chain in its bank is complete (sem_pe gates below).
# Host pre-scaling folds every Gram coefficient into C_ZZ so chains can
# share psum regions (fewer diag reductions): gathered rz rows carry -2*rz
# (so gz tiles sum to -2*sum(g.z) under C_ZZ, and gg tiles to 4*sum(g^2)
# under C_ZZ/4), and qv carries 0.4*qy*V (C_QL = 0.4*C_ZZ; the ln recovers
# the unscaled argument via scale=2.5).
_REGIONS = [
    ("mainA", 0, C_ZZ),  # bank 0: ql + zz(b0-3) + gz'(b0-3)
    ("gg", 512, C_ZZ / 4),  # bank 1: gg'(b0-7)
    ("mainB", 1024, C_ZZ),  # bank 2: zz+gz'(b4-7), the last chain
]
REG_OFF = {n: o for n, o, _ in _REGIONS}
REG_COEF = {n: c for n, _, c in _REGIONS}
NPS_ALLOC = 1536

# acc columns: 3 diags + sqA + sqB + best
NACC = 6

# cpack layout (f32 cols): 0:16 GA idx, 16:32 GB idx, 32 ln bias,
# 33:49 w*best, 49:65 w*best_gt, 65:128 pad
NCONST = 128
BC = BL * C  # 16

_CACHE: dict = {}


def _build_bass(vector_dims: int):
    import concourse.bacc as bacc
    import concourse.bass as bass
    from concourse import mybir

    f32 = mybir.dt.float32
    f8e4 = mybir.dt.float8e4
    bf = mybir.dt.bfloat16
    u8 = mybir.dt.uint8
    i16 = mybir.dt.int16
    i32 = mybir.dt.int32
    Act = mybir.ActivationFunctionType
    Alu = mybir.AluOpType
    DR = mybir.MatmulPerfMode.DoubleRow

    nc = bacc.Bacc("TRN2", target_bir_lowering=False)

    zs = nc.dram_tensor("zs", [BL * S, D], f8e4, kind="ExternalInput")
    gath = nc.dram_tensor("gath", [BL * S, K], u8, kind="ExternalInput")
    ptsgt = nc.dram_tensor("ptsgt", [S, BL * PC], f8e4, kind="ExternalInput")
    qv = nc.dram_tensor("qv", [S, BL * V], f8e4, kind="ExternalInput")
    cpack = nc.dram_tensor("cpack", [S, NCONST], f32, kind="ExternalInput")
    po = nc.dram_tensor("po", [S, NACC], f32, kind="ExternalOutput")

    from contextlib import ExitStack

    with ExitStack() as ctx:
        zs_t = ctx.enter_context(nc.sbuf_tensor([S, BL * D], f8e4))
        gt_t = ctx.enter_context(nc.sbuf_tensor([S, BL * K], u8))
        qy_t = ctx.enter_context(nc.sbuf_tensor([S, BL * V], f8e4))
        lq_t = ctx.enter_context(nc.sbuf_tensor([S, BL * V], f8e4))
        pg_t = ctx.enter_context(nc.sbuf_tensor([S, BL * PC], f8e4))
        pd_t = ctx.enter_context(nc.sbuf_tensor([S, BL * PC], bf))
        cp_t = ctx.enter_context(nc.sbuf_tensor([S, NCONST], f32))
        pm_t = ctx.enter_context(nc.sbuf_tensor([S, S], i32))
        id_t = ctx.enter_context(nc.sbuf_tensor([S, S], f32))
        ctx_t = ctx.enter_context(nc.sbuf_tensor([S, 1], i32))
        bd_t = ctx.enter_context(nc.sbuf_tensor([S, BC], f32))
        acc_t = ctx.enter_context(nc.sbuf_tensor([S, NACC], f32))
        ps_t = ctx.enter_context(nc.psum_tensor([S, NPS_ALLOC], f32))

        sem_cp = ctx.enter_context(nc.semaphore("sem_cp"))
        sem_qv = ctx.enter_context(nc.semaphore("sem_qv"))
        sem_zs = [
            ctx.enter_context(nc.semaphore(f"sem_zs{c}")) for c in range(9)
        ]
        sem_gA = ctx.enter_context(nc.semaphore("sem_gA"))
        sem_gB = ctx.enter_context(nc.semaphore("sem_gB"))
        sem_pt = ctx.enter_context(nc.semaphore("sem_pt"))
        sem_io = ctx.enter_context(nc.semaphore("sem_io"))
        sem_prep = ctx.enter_context(nc.semaphore("sem_prep"))
        sem_trig = ctx.enter_context(nc.semaphore("sem_trig"))
        sem_act = ctx.enter_context(nc.semaphore("sem_act"))
        sem_dve = ctx.enter_context(nc.semaphore("sem_dve"))
        sem_pe = ctx.enter_context(nc.semaphore("sem_pe"))
        sem_out = ctx.enter_context(nc.semaphore("sem_out"))
        block = ctx.enter_context(nc.Block())

        cp16 = cp_t[:].bitcast(i16)  # [S, 256] i16
        gt3 = gt_t[:].rearrange("s (b k) -> s b k", b=BL)
        pg3 = pg_t[:].rearrange("s (b p) -> s b p", b=BL)  # p = PC dense
        pd3 = pd_t[:].rearrange("s (b p) -> s b p", b=BL)

        def sup(ap):  # 256-col slice -> DoubleRow [s, 2, 128] view
            return ap.rearrange("s (j m) -> s j m", j=2)

        def z_sup(b, t):
            o = b * D + t * 256
            return sup(zs_t[:, o : o + 256])

        def g_sup(b, t):
            o = b * K + t * 256
            return sup(gt_t[:, o : o + 256].bitcast(f8e4))

        def q_sup(i):
            return sup(qy_t[:, i * 256 : (i + 1) * 256])

        def l_sup(i):
            return sup(lq_t[:, i * 256 : (i + 1) * 256])

        @block.sync
        def _(sync):
            sync.dma_start(out=qy_t[:], in_=qv[:]).then_inc(sem_qv, 16)
            sync.dma_start(out=cp_t[:], in_=cpack[:]).then_inc(sem_cp, 16)
            sync.dma_start(
                out=zs_t[:, 0 : 2 * D], in_=zs[0 : 2 * S, :]
            ).then_inc(sem_zs[0], 16)
            sync.dma_start(
                out=zs_t[:, 2 * D : 4 * D], in_=zs[2 * S : 4 * S, :]
            ).then_inc(sem_zs[1], 16)
            # hold the late loads until both gathers are triggered so the
            # gathers win DMA-device arbitration
            sync.wait_ge(sem_trig, 1)
            sync.dma_start(out=pg_t[:], in_=ptsgt[:]).then_inc(sem_pt, 16)
            sync.dma_start(
                out=zs_t[:, 4 * D : 5 * D], in_=zs[4 * S : 5 * S, :]
            ).then_inc(sem_zs[2], 16)
            sync.dma_start(
                out=zs_t[:, 5 * D : 6 * D], in_=zs[5 * S : 6 * S, :]
            ).then_inc(sem_zs[3], 16)
            sync.dma_start(
                out=zs_t[:, 6 * D : 7 * D], in_=zs[6 * S : 7 * S, :]
            ).then_inc(sem_zs[4], 16)
            for q in range(4):
                sync.dma_start(
                    out=zs_t[:, 7 * D + 512 * q : 7 * D + 512 * (q + 1)],
                    in_=zs[7 * S : 8 * S, 512 * q : 512 * (q + 1)],
                ).then_inc(sem_zs[5 + q], 16)

        @block.gpsimd
        def _(gpsimd):
            # identity basis (p - f) and zero ctx idxs, both iota (standard
            # lib; Bacc inserts the attnmlp library load before the preps)
            gpsimd.iota(
                out=pm_t[:], pattern=[[-1, S]], base=0, channel_multiplier=1
            ).then_inc(sem_io, 1)
            gpsimd.iota(
                out=ctx_t[:], pattern=[[0, 1]], base=0, channel_multiplier=0
            ).then_inc(sem_io, 1)
            gpsimd.wait_ge(sem_io, 2)
            gpsimd.wait_ge(sem_cp, 16)
            # batched gathers: 4 batches each, idx tables in cpack
            gpsimd.dma_gather(
                out_ap=gt3[:, 0:4, :],
                in_ap=gath[:],
                idxs_ap=cp16[:, 0:32],
                num_idxs=4 * S,
                num_idxs_reg=4 * S,
                elem_size=K,
                prepare_only=True,
                sem=sem_gA,
            ).then_inc(sem_prep, 1)
            gpsimd.wait_ge(sem_prep, 1)
            gpsimd.trigger_dma(count=1)
            gpsimd.dma_gather(
                out_ap=gt3[:, 4:8, :],
                in_ap=gath[:],
                idxs_ap=cp16[:, 32:64],
                num_idxs=4 * S,
                num_idxs_reg=4 * S,
                elem_size=K,
                prepare_only=True,
                sem=sem_gB,
            ).then_inc(sem_prep, 1)
            gpsimd.wait_ge(sem_prep, 2)
            gpsimd.trigger_dma(count=1)
            gpsimd.sem_inc(sem_trig, 1)
            # output writeback: prep once gather A has drained the SWDGE ring
            # (keeps outstanding descriptors within the 16 KiB carveout),
            # fire after the last accumulation
            gpsimd.wait_ge(sem_gA, 16)
            gpsimd.kv_writeback(
                out_ap=po[:].rearrange("(a p) (o n) -> a p o n", a=1, o=1),
                in_ap=acc_t[:].rearrange("p (o b n) -> p o b n", o=1, b=1),
                ctx_idxs_ap=ctx_t[:],
                prepare_only=True,
                sem=sem_out,
            ).then_inc(sem_prep, 1)
            gpsimd.wait_ge(sem_prep, 3)
            gpsimd.wait_ge(sem_act, 4)  # ln + bd^2 + sqA + sqB (early)
            gpsimd.wait_ge(sem_dve, 7)  # all diags done (the late gate)
            gpsimd.trigger_dma(count=1)

        @block.tensor
        def _(tensor):
            def mm(region, lhsT, rhs, start, stop):
                o = REG_OFF[region]
                return nc.tensor.matmul(
                    out=ps_t[:, o : o + 128],
                    lhsT=lhsT, rhs=rhs, start=start, stop=stop,
                    perf_mode=DR, skip_group_check=True,
                )

            def emit(region, tiles, waits_at=None, inc=False):
                n = len(tiles)
                for i, (lhsT, rhs) in enumerate(tiles):
                    if waits_at and i in waits_at:
                        for semh, val in waits_at[i]:
                            tensor.wait_ge(semh, val)
                    m = mm(region, lhsT, rhs, start=(i == 0), stop=(i == n - 1))
                if inc:
                    m.then_inc(sem_pe, 1)

            def zz_tiles(bs, ts=None):
                ts = ts if ts is not None else range(NDT)
                return [(z_sup(b, t), z_sup(b, t)) for b in bs for t in ts]

            def gz_tiles(bs):
                return [(g_sup(b, t), z_sup(b, t)) for b in bs for t in range(NDT)]

            def gz_tiles_r(bs, ts):
                return [(g_sup(b, t), z_sup(b, t)) for b in bs for t in ts]

            def gg_tiles(bs):
                return [(g_sup(b, t), g_sup(b, t)) for b in bs for t in range(NDT)]

            ql_tiles = [(q_sup(i), l_sup(i)) for i in range(BL * V // 256)]
            # mainA: zz(b0,b1) | ql | zz(b2,b3) | gz'(b0-3)  (one psum chain;
            # tile order within an accumulation group is free)
            emit(
                "mainA",
                zz_tiles((0, 1)) + ql_tiles + zz_tiles((2, 3))
                + gz_tiles((0, 1, 2, 3)),
                waits_at={
                    0: [(sem_zs[0], 16)],
                    16: [(sem_qv, 16), (sem_act, 1)],
                    32: [(sem_zs[1], 16)],
                    48: [(sem_gA, 16)],
                },
                inc=True,  # pe 1: bank 0 done
            )
            emit(
                "gg",
                gg_tiles((0, 1, 2, 3)) + gg_tiles((4, 5, 6, 7)),
                waits_at={32: [(sem_gB, 16)]},
                inc=True,  # pe 2: bank 1 done
            )
            # mainB: zz+gz' for b4-7, gated per zs chunk; b7 col-split so only
            # 4 matmuls trail the last 512B transfer
            mb_tiles = []
            mb_waits = {}
            for seg, (bs, ts, sems) in enumerate(
                [((4,), range(NDT), [sem_zs[2], sem_gB]),
                 ((5,), range(NDT), [sem_zs[3]]),
                 ((6,), range(NDT), [sem_zs[4]]),
                 ((7,), (0, 1), [sem_zs[5]]),
                 ((7,), (2, 3), [sem_zs[6]]),
                 ((7,), (4, 5), [sem_zs[7]]),
                 ((7,), (6, 7), [sem_zs[8]])]
            ):
                mb_waits[len(mb_tiles)] = [(sm, 16) for sm in sems]
                mb_tiles += zz_tiles(bs, ts) + gz_tiles_r(bs, ts)
            emit("mainB", mb_tiles, waits_at=mb_waits, inc=True)  # pe 3

        # sem_dve increments, in DVE program order:
        #  1 id | 2 bd | 3 mainA diag | 4 pdA | 5 pdB | 6 gg diag
        #  7 mainB diag
        # sem_act: 1 ln | 2 bd^2 | 3 sqA | 4 sqB
        @block.vector
        def _(vector):
            # identity tile from the iota (p - f == 0)
            vector.wait_ge(sem_io, 1)
            nc.vector.tensor_scalar(
                out=id_t[:], in0=pm_t[:], scalar1=0, scalar2=None,
                op0=Alu.is_equal,
            ).then_inc(sem_dve, 1)
            # best diff (rows >= P are zero in cpack -> contribute 0)
            vector.wait_ge(sem_cp, 16)
            nc.vector.tensor_sub(
                bd_t[:], cp_t[:, 33 : 33 + BC], cp_t[:, 49 : 49 + BC]
            ).then_inc(sem_dve, 1)

            def diag(region, col):
                o = REG_OFF[region]
                nc.vector.scalar_tensor_tensor(
                    out=ps_t[:, o : o + 128],
                    in0=ps_t[:, o : o + 128],
                    scalar=float(REG_COEF[region]),
                    in1=id_t[:],
                    op0=Alu.mult, op1=Alu.mult,
                    accum_out=acc_t[:, col : col + 1],
                ).then_inc(sem_dve, 1)

            vector.wait_ge(sem_dve, 1)  # id_t engine-write visible
            vector.wait_ge(sem_pe, 1)
            diag("mainA", 0)
            # pts diffs (gathered x already weighted; y = weighted gt)
            vector.wait_ge(sem_pt, 16)
            vector.wait_ge(sem_gA, 16)
            nc.vector.tensor_sub(
                pd3[:, 0:4, :],
                gt3[:, 0:4, D : D + PC].bitcast(f8e4),
                pg3[:, 0:4, :],
            ).then_inc(sem_dve, 1)
            vector.wait_ge(sem_gB, 16)
            nc.vector.tensor_sub(
                pd3[:, 4:8, :],
                gt3[:, 4:8, D : D + PC].bitcast(f8e4),
                pg3[:, 4:8, :],
            ).then_inc(sem_dve, 1)
            vector.wait_ge(sem_pe, 2)
            diag("gg", 1)
            vector.wait_ge(sem_pe, 3)
            diag("mainB", 2)

        @block.scalar
        def _(scalar):
            scalar.wait_ge(sem_qv, 16)
            scalar.wait_ge(sem_cp, 16)
            nc.scalar.activation(
                lq_t[:], qy_t[:], Act.Ln, bias=cp_t[:, 32:33], scale=2.5
            ).then_inc(sem_act, 1)
            scalar.wait_ge(sem_dve, 2)  # bd diff done
            nc.scalar.activation(
                bd_t[:], bd_t[:], Act.Square, accum_out=acc_t[:, 5:6]
            ).then_inc(sem_act, 1)
            scalar.wait_ge(sem_dve, 4)  # pd_A diff done
            nc.scalar.activation(
                pd3[:, 0:4, :], pd3[:, 0:4, :], Act.Square,
                accum_out=acc_t[:, 3:4],
            ).then_inc(sem_act, 1)
            scalar.wait_ge(sem_dve, 5)  # pd_B diff done
            nc.scalar.activation(
                pd3[:, 4:8, :], pd3[:, 4:8, :], Act.Square,
                accum_out=acc_t[:, 4:5],
            ).then_inc(sem_act, 1)

    nc.compile()
    return nc


def _get_nc(vector_dims: int):
    key = ("nc", vector_dims)
    if key not in _CACHE:
        _CACHE[key] = _build_bass(vector_dims)
    return _CACHE[key]


def _pack_idx(idxs):
    """int idx array (n % 16 == 0) -> [128, n/32] f32 idx table
    (wrap-16, replicated to 128 partitions)."""
    idxs = np.asarray(idxs, dtype=np.int16)
    n = len(idxs)
    t = idxs.reshape(n // 16, 16).T  # [16, n/16]
    t = np.tile(t, (8, 1))  # [128, n/16]
    f = np.zeros((128, n // 32), dtype=np.float32)
    f.view(np.int16)[:] = t
    return f


def _prepare(inputs):
    import ml_dtypes

    e4 = ml_dtypes.float8_e4m3

    zs = np.asarray(inputs["zs"], dtype=np.float32)
    rzs = np.asarray(inputs["rzs"], dtype=np.float32)
    pts = np.asarray(inputs["pts"], dtype=np.float32)
    pts_gt = np.asarray(inputs["pts_gt"], dtype=np.float32)
    qy = np.asarray(inputs["qy"], dtype=np.float32)
    best = np.asarray(inputs["best"], dtype=np.float64)
    best_gt = np.asarray(inputs["best_gt"], dtype=np.float64)
    mapping = np.asarray(inputs["mapping"])
    vector_dims = int(np.asarray(inputs["vector_dims"]))

    w_p = np.ones(P, dtype=np.float64)
    w_p[list(MARKS)] += W_MARK
    w_sq = np.sqrt(w_p)
    wc = w_sq[None, None, :, None]

    zs_q = np.ascontiguousarray(zs.astype(e4))
    qv_q = (qy * np.float32(0.4 * vector_dims)).astype(e4)

    wpts_q = np.zeros((B, S, PCP), dtype=e4)
    wpts_q[:, :, :PC] = (pts * wc).astype(np.float32).astype(e4).reshape(B, S, PC)
    ptsgt_q = (pts_gt * wc).astype(np.float32).astype(e4).reshape(B, S, PC)

    gath_b = np.empty((B, S, K), dtype=np.uint8)
    gath_b[:, :, :D] = (np.float32(-2.0) * rzs).astype(e4).view(np.uint8)
    gath_b[:, :, D:] = wpts_q.view(np.uint8)

    best_w = (best * w_sq[None, :, None]).astype(np.float32)
    bestgt_w = (best_gt * w_sq[None, :, None]).astype(np.float32)

    in_maps = []
    for c in range(N_CORES):
        sl = slice(c * BL, (c + 1) * BL)
        map_c = mapping[sl].astype(np.int32)  # [BL, S]
        # gather token k = b_local*128 + s -> absolute row b_local*S + map
        idx_all = (
            np.arange(BL)[:, None] * S + map_c
        ).reshape(BL * S).astype(np.int16)
        cpk = np.zeros((S, NCONST), dtype=np.float32)
        cpk[:, 0:16] = _pack_idx(idx_all[0 : 4 * S])
        cpk[:, 16:32] = _pack_idx(idx_all[4 * S : 8 * S])
        cpk[:, 32] = np.float32(LN_B0)
        cpk[:P, 33 : 33 + BC] = best_w[sl].transpose(1, 0, 2).reshape(P, BC)
        cpk[:P, 49 : 49 + BC] = bestgt_w[sl].transpose(1, 0, 2).reshape(P, BC)
        in_maps.append(
            {
                "zs": zs_q[sl].reshape(BL * S, D),
                "gath": gath_b[sl].reshape(BL * S, K),
                "ptsgt": np.ascontiguousarray(
                    ptsgt_q[sl].transpose(1, 0, 2).reshape(S, BL * PC)
                ),
                "qv": np.ascontiguousarray(
                    qv_q[sl].transpose(1, 0, 2).reshape(S, BL * V)
                ),
                "cpack": cpk,
            }
        )
    return in_maps, vector_dims


def _combine(results) -> np.ndarray:
    total = np.float64(0.0)
    for r in results:
        por = r["po"].astype(np.float64)
        total += (
            por[:, 0:3].sum()
            + por[:, 3:5].sum() / (B * S * PC)
            + por[:, 5].sum() / (B * PC)
        )
    return np.float32(total)


def kernel(**inputs) -> np.ndarray:
    from concourse.bass_utils import run_bass_kernel_spmd

    in_maps, vector_dims = _prepare(inputs)
    nc = _get_nc(vector_dims)

    trace = os.environ.get("KERNEL_TRACE", "") == "1"
    res = run_bass_kernel_spmd(nc, in_maps, core_ids=list(range(N_CORES)), trace=trace)
    if trace and res.exec_time_ns is not None:
        print(f"HW exec time: {res.exec_time_ns} ns")

    return _combine(res.results)


# revision 9
# speedup vs baseline: 1.0010x; 1.0010x over previous
"""Trainium2 Bass kernel for nn_CQLoss (composite loss function).

Strategy: pure data parallel over batch dim (64 batches -> 8 per core), all
large tensors travelling as fp8-e4m3. Every loss term is expanded into global
sums of products computed on the PE as PSUM-accumulated DoubleRow Gram-tile
chains (diag of psum += tile^T @ tile' holds the per-column dot products):

  recon*N  = sum g^2 - 2 sum g.z + sum z^2      (g = mapping-gathered rzs)
  pts*N    = host-weighted (x - y)^2 via DVE diff + ACT square-accumulate
  kld*N*V  = sum qV * ln(qV + 2^-9)  (PE: qV (x) ln-tile diag)
  best*N   = subtract/square (tiny, f32)

Host pre-scaling folds every Gram coefficient into C_ZZ (gather rows carry
-2*rz; qv carries 0.4*qy*V with the ln recovering via scale=2.5) so all 144
z/gz/ql tiles accumulate into just THREE psum regions (mainA, gg, mainB) --
three masked-diag reductions total, exactly one of which (mainB, 258 ns)
sits in the end-of-kernel tail.

DMA architecture (the kernel is DMA-bound; the cost model serializes all
transfers on one 360 B/ns device, so wall-clock ~= first-transfer latency +
total transfer time + last-transfer dependent tail):
  - mapping-gathered rows ride in TWO batched SWDGE dma_gather ops (4 batches
    each, 512 rows x 2304B), prepared on the Pool engine from i16 idx tables
    uploaded in cpack and fired by trigger_dma -- triggered transfers skip
    the HWDGE-gen and DGE-delay pipeline stages.
  - direct loads (qv, cpack, zs chunks, ptsgt) are SP-issued HWDGE copies:
    qv first (its 1.46 us transfer hides the later gens' pipeline), gathers
    mid-stream, then ptsgt and the remaining zs batches; batch 7's zs is
    split into four 512B column-slices (512B is the smallest descriptor that
    still runs at full bus rate) so the last transfer gates only 4 matmuls.
  - the scalar output leaves via a kv_writeback prepared mid-kernel (after
    gather A drains the SWDGE ring) and triggered right after the last
    accumulation, collapsing the output pipeline to trigger+transfer+sem;
    no engine waits on the output semaphore -- the runtime's own completion
    barrier covers it.
The remaining tail is structural: 900 ns DMA-sem propagation on the last
input, ~160 ns PE, ~440 ns DVE diag (incl dispatch), ~220 ns Pool trigger
hops, and 900 ns output-sem propagation.

PSUM: mainA/gg/mainB each own a 128-col region in its own bank (the DVE
must not read a psum bank the PE still writes; matmul accumulation chains
must stay contiguous in the PE stream -- tile order WITHIN a chain is free,
which is what lets one region span many waits). Each diag reduction is one
scalar_tensor_tensor (psum * coef * identity, accumulated into one acc
column); the identity tile is built on-chip from a Pool iota (p - f) and a
DVE is_equal. The host sums partitions/cores in float64.
"""

import os
import sys

import numpy as np

for _p in ("/opt/trn_rl_repo", "/root/.axon_site/_ro/trn_rl_repo"):
    if os.path.isdir(_p) and _p not in sys.path:
        sys.path.insert(0, _p)

B, S, D, P, C, V = 64, 128, 2048, 118, 2, 512
PC = P * C  # 236
PCP = 256  # padded pts width
K = D + PCP  # gather row bytes: 2304
N_CORES = 8
BL = B // N_CORES  # 8 batches per core
ALPHA, BETA, GAMMA, EPS = 10.0, 0.1, 1.0, 1e-20
MARKS = (0, 29, 88, 117)
W_MARK = ALPHA * PC / (len(MARKS) * C)  # 295.0
LN_B0 = 2.0 ** -9

# final linear-combination coefficients (applied via the psum diag masks)
C_ZZ = GAMMA / (B * S * D)
C_GZ = -2.0 * GAMMA / (B * S * D)
C_QL = BETA / (B * S * V)

NDT = D // 256  # 8 DoubleRow supertiles per batch

# psum regions: (name, bank-ordered col offset, coefficient)
# banks (512 cols) grouped by chain completion; diag of a region only runs
# after every chain in its bank is complete (sem_pe gates below).
# Host pre-scaling folds every Gram coefficient into C_ZZ so chains can
# share psum regions (fewer diag reductions): gathered rz rows carry -2*rz
# (so gz tiles sum to -2*sum(g.z) under C_ZZ, and gg tiles to 4*sum(g^2)
# under C_ZZ/4), and qv carries 0.4*qy*V (C_QL = 0.4*C_ZZ; the ln recovers
# the unscaled argument via scale=2.5).
_REGIONS = [
    ("mainA", 0, C_ZZ),  # bank 0: ql + zz(b0-3) + gz'(b0-3)
    ("gg", 512, C_ZZ / 4),  # bank 1: gg'(b0-7)
    ("mainB", 1024, C_ZZ),  # bank 2: zz+gz'(b4-7), the last chain
]
REG_OFF = {n: o for n, o, _ in _REGIONS}
REG_COEF = {n: c for n, _, c in _REGIONS}
NPS_ALLOC = 1536

# acc columns: 3 diags + sqA + sqB + best
NACC = 6

# cpack layout (f32 cols): 0:16 GA idx, 16:32 GB idx, 32 ln bias,
# 33:49 w*best, 49:65 w*best_gt, 65:128 pad
NCONST = 128
BC = BL * C  # 16

_CACHE: dict = {}


def _build_bass(vector_dims: int):
    import concourse.bacc as bacc
    import concourse.bass as bass
    from concourse import mybir

    f32 = mybir.dt.float32
    f8e4 = mybir.dt.float8e4
    bf = mybir.dt.bfloat16
    u8 = mybir.dt.uint8
    i16 = mybir.dt.int16
    i32 = mybir.dt.int32
    Act = mybir.ActivationFunctionType
    Alu = mybir.AluOpType
    DR = mybir.MatmulPerfMode.DoubleRow

    nc = bacc.Bacc("TRN2", target_bir_lowering=False,
                   dynamic_dma_scratch_size=32768)

    zs = nc.dram_tensor("zs", [BL * S, D], f8e4, kind="ExternalInput")
    gath = nc.dram_tensor("gath", [BL * S, K], u8, kind="ExternalInput")
    ptsgt = nc.dram_tensor("ptsgt", [S, BL * PC], f8e4, kind="ExternalInput")
    qv = nc.dram_tensor("qv", [S, BL * V], f8e4, kind="ExternalInput")
    cpack = nc.dram_tensor("cpack", [S, NCONST], f32, kind="ExternalInput")
    po = nc.dram_tensor("po", [S, NACC], f32, kind="ExternalOutput")

    from contextlib import ExitStack

    with ExitStack() as ctx:
        zs_t = ctx.enter_context(nc.sbuf_tensor([S, BL * D], f8e4))
        gt_t = ctx.enter_context(nc.sbuf_tensor([S, BL * K], u8))
        qy_t = ctx.enter_context(nc.sbuf_tensor([S, BL * V], f8e4))
        lq_t = ctx.enter_context(nc.sbuf_tensor([S, BL * V], f8e4))
        pg_t = ctx.enter_context(nc.sbuf_tensor([S, BL * PC], f8e4))
        pd_t = ctx.enter_context(nc.sbuf_tensor([S, BL * PC], bf))
        cp_t = ctx.enter_context(nc.sbuf_tensor([S, NCONST], f32))
        pm_t = ctx.enter_context(nc.sbuf_tensor([S, S], i32))
        id_t = ctx.enter_context(nc.sbuf_tensor([S, S], f32))
        ctx_t = ctx.enter_context(nc.sbuf_tensor([S, 1], i32))
        bd_t = ctx.enter_context(nc.sbuf_tensor([S, BC], f32))
        acc_t = ctx.enter_context(nc.sbuf_tensor([S, NACC], f32))
        ps_t = ctx.enter_context(nc.psum_tensor([S, NPS_ALLOC], f32))

        sem_cp = ctx.enter_context(nc.semaphore("sem_cp"))
        sem_qv = ctx.enter_context(nc.semaphore("sem_qv"))
        sem_zs = [
            ctx.enter_context(nc.semaphore(f"sem_zs{c}")) for c in range(9)
        ]
        sem_gA = ctx.enter_context(nc.semaphore("sem_gA"))
        sem_gB = ctx.enter_context(nc.semaphore("sem_gB"))
        sem_pt = ctx.enter_context(nc.semaphore("sem_pt"))
        sem_io = ctx.enter_context(nc.semaphore("sem_io"))
        sem_prep = ctx.enter_context(nc.semaphore("sem_prep"))
        sem_act = ctx.enter_context(nc.semaphore("sem_act"))
        sem_dve = ctx.enter_context(nc.semaphore("sem_dve"))
        sem_pe = ctx.enter_context(nc.semaphore("sem_pe"))
        sem_out = ctx.enter_context(nc.semaphore("sem_out"))
        block = ctx.enter_context(nc.Block())

        cp16 = cp_t[:].bitcast(i16)  # [S, 256] i16
        gt3 = gt_t[:].rearrange("s (b k) -> s b k", b=BL)
        pg3 = pg_t[:].rearrange("s (b p) -> s b p", b=BL)  # p = PC dense
        pd3 = pd_t[:].rearrange("s (b p) -> s b p", b=BL)

        def sup(ap):  # 256-col slice -> DoubleRow [s, 2, 128] view
            return ap.rearrange("s (j m) -> s j m", j=2)

        def z_sup(b, t):
            o = b * D + t * 256
            return sup(zs_t[:, o : o + 256])

        def g_sup(b, t):
            o = b * K + t * 256
            return sup(gt_t[:, o : o + 256].bitcast(f8e4))

        def q_sup(i):
            return sup(qy_t[:, i * 256 : (i + 1) * 256])

        def l_sup(i):
            return sup(lq_t[:, i * 256 : (i + 1) * 256])

        @block.sync
        def _(sync):
            sync.dma_start(out=qy_t[:], in_=qv[:]).then_inc(sem_qv, 16)
            sync.dma_start(out=cp_t[:], in_=cpack[:]).then_inc(sem_cp, 16)
            sync.dma_start(
                out=zs_t[:, 0 : 2 * D], in_=zs[0 : 2 * S, :]
            ).then_inc(sem_zs[0], 16)
            sync.dma_start(
                out=zs_t[:, 2 * D : 4 * D], in_=zs[2 * S : 4 * S, :]
            ).then_inc(sem_zs[1], 16)
            # hold the late loads until both gather preps are done (the
            # triggers follow within ~100ns, and the late loads' own
            # gen+DGE pipeline adds ~1.9us) so the gathers win DMA-device
            # arbitration. NOTE: do not gate on a sem_inc placed after a
            # trigger_dma -- Bacc fuses the inc onto the trigger, whose
            # updates only fire after it acquires the DMA device + 900ns.
            sync.wait_ge(sem_prep, 2)
            sync.dma_start(out=pg_t[:], in_=ptsgt[:]).then_inc(sem_pt, 16)
            sync.dma_start(
                out=zs_t[:, 4 * D : 5 * D], in_=zs[4 * S : 5 * S, :]
            ).then_inc(sem_zs[2], 16)
            sync.dma_start(
                out=zs_t[:, 5 * D : 6 * D], in_=zs[5 * S : 6 * S, :]
            ).then_inc(sem_zs[3], 16)
            sync.dma_start(
                out=zs_t[:, 6 * D : 7 * D], in_=zs[6 * S : 7 * S, :]
            ).then_inc(sem_zs[4], 16)
            for q in range(4):
                sync.dma_start(
                    out=zs_t[:, 7 * D + 512 * q : 7 * D + 512 * (q + 1)],
                    in_=zs[7 * S : 8 * S, 512 * q : 512 * (q + 1)],
                ).then_inc(sem_zs[5 + q], 16)

        @block.gpsimd
        def _(gpsimd):
            # identity basis (p - f) and zero ctx idxs, both iota (standard
            # lib; Bacc inserts the attnmlp library load before the preps)
            gpsimd.iota(
                out=pm_t[:], pattern=[[-1, S]], base=0, channel_multiplier=1
            ).then_inc(sem_io, 1)
            gpsimd.iota(
                out=ctx_t[:], pattern=[[0, 1]], base=0, channel_multiplier=0
            ).then_inc(sem_io, 1)
            gpsimd.wait_ge(sem_io, 2)
            gpsimd.wait_ge(sem_cp, 16)
            # batched gathers: 4 batches each, idx tables in cpack
            gpsimd.dma_gather(
                out_ap=gt3[:, 0:4, :],
                in_ap=gath[:],
                idxs_ap=cp16[:, 0:32],
                num_idxs=4 * S,
                num_idxs_reg=4 * S,
                elem_size=K,
                prepare_only=True,
                sem=sem_gA,
            ).then_inc(sem_prep, 1)
            gpsimd.wait_ge(sem_prep, 1)
            gpsimd.trigger_dma(count=1)
            gpsimd.dma_gather(
                out_ap=gt3[:, 4:8, :],
                in_ap=gath[:],
                idxs_ap=cp16[:, 32:64],
                num_idxs=4 * S,
                num_idxs_reg=4 * S,
                elem_size=K,
                prepare_only=True,
                sem=sem_gB,
            ).then_inc(sem_prep, 1)
            gpsimd.wait_ge(sem_prep, 2)
            gpsimd.trigger_dma(count=1)
            # output writeback: prep now (32 KiB carveout holds both
            # gathers + this), fire after the last accumulation
            gpsimd.kv_writeback(
                out_ap=po[:].rearrange("(a p) (o n) -> a p o n", a=1, o=1),
                in_ap=acc_t[:].rearrange("p (o b n) -> p o b n", o=1, b=1),
                ctx_idxs_ap=ctx_t[:],
                prepare_only=True,
                sem=sem_out,
            ).then_inc(sem_prep, 1)
            gpsimd.wait_ge(sem_prep, 3)
            gpsimd.wait_ge(sem_act, 4)  # ln + bd^2 + sqA + sqB (early)
            gpsimd.wait_ge(sem_dve, 7)  # all diags done (the late gate)
            gpsimd.trigger_dma(count=1)

        @block.tensor
        def _(tensor):
            def mm(region, lhsT, rhs, start, stop):
                o = REG_OFF[region]
                return nc.tensor.matmul(
                    out=ps_t[:, o : o + 128],
                    lhsT=lhsT, rhs=rhs, start=start, stop=stop,
                    perf_mode=DR, skip_group_check=True,
                )

            def emit(region, tiles, waits_at=None, inc=False):
                n = len(tiles)
                for i, (lhsT, rhs) in enumerate(tiles):
                    if waits_at and i in waits_at:
                        for semh, val in waits_at[i]:
                            tensor.wait_ge(semh, val)
                    m = mm(region, lhsT, rhs, start=(i == 0), stop=(i == n - 1))
                if inc:
                    m.then_inc(sem_pe, 1)

            def zz_tiles(bs, ts=None):
                ts = ts if ts is not None else range(NDT)
                return [(z_sup(b, t), z_sup(b, t)) for b in bs for t in ts]

            def gz_tiles(bs):
                return [(g_sup(b, t), z_sup(b, t)) for b in bs for t in range(NDT)]

            def gz_tiles_r(bs, ts):
                return [(g_sup(b, t), z_sup(b, t)) for b in bs for t in ts]

            def gg_tiles(bs):
                return [(g_sup(b, t), g_sup(b, t)) for b in bs for t in range(NDT)]

            ql_tiles = [(q_sup(i), l_sup(i)) for i in range(BL * V // 256)]
            # mainA: zz(b0,b1) | ql | zz(b2,b3) | gz'(b0-3)  (one psum chain;
            # tile order within an accumulation group is free)
            emit(
                "mainA",
                zz_tiles((0, 1)) + ql_tiles + zz_tiles((2, 3))
                + gz_tiles((0, 1, 2, 3)),
                waits_at={
                    0: [(sem_zs[0], 16)],
                    16: [(sem_qv, 16), (sem_act, 1)],
                    32: [(sem_zs[1], 16)],
                    48: [(sem_gA, 16)],
                },
                inc=True,  # pe 1: bank 0 done
            )
            emit(
                "gg",
                gg_tiles((0, 1, 2, 3)) + gg_tiles((4, 5, 6, 7)),
                waits_at={32: [(sem_gB, 16)]},
                inc=True,  # pe 2: bank 1 done
            )
            # mainB: zz+gz' for b4-7, gated per zs chunk; b7 col-split so only
            # 4 matmuls trail the last 512B transfer
            mb_tiles = []
            mb_waits = {}
            for seg, (bs, ts, sems) in enumerate(
                [((4,), range(NDT), [sem_zs[2], sem_gB]),
                 ((5,), range(NDT), [sem_zs[3]]),
                 ((6,), range(NDT), [sem_zs[4]]),
                 ((7,), (0, 1), [sem_zs[5]]),
                 ((7,), (2, 3), [sem_zs[6]]),
                 ((7,), (4, 5), [sem_zs[7]]),
                 ((7,), (6, 7), [sem_zs[8]])]
            ):
                mb_waits[len(mb_tiles)] = [(sm, 16) for sm in sems]
                mb_tiles += zz_tiles(bs, ts) + gz_tiles_r(bs, ts)
            emit("mainB", mb_tiles, waits_at=mb_waits, inc=True)  # pe 3

        # sem_dve increments, in DVE program order:
        #  1 id | 2 bd | 3 mainA diag | 4 pdA | 5 pdB | 6 gg diag
        #  7 mainB diag
        # sem_act: 1 ln | 2 bd^2 | 3 sqA | 4 sqB
        @block.vector
        def _(vector):
            # identity tile from the iota (p - f == 0)
            vector.wait_ge(sem_io, 1)
            nc.vector.tensor_scalar(
                out=id_t[:], in0=pm_t[:], scalar1=0, scalar2=None,
                op0=Alu.is_equal,
            ).then_inc(sem_dve, 1)
            # best diff (rows >= P are zero in cpack -> contribute 0)
            vector.wait_ge(sem_cp, 16)
            nc.vector.tensor_sub(
                bd_t[:], cp_t[:, 33 : 33 + BC], cp_t[:, 49 : 49 + BC]
            ).then_inc(sem_dve, 1)

            def diag(region, col):
                o = REG_OFF[region]
                nc.vector.scalar_tensor_tensor(
                    out=ps_t[:, o : o + 128],
                    in0=ps_t[:, o : o + 128],
                    scalar=float(REG_COEF[region]),
                    in1=id_t[:],
                    op0=Alu.mult, op1=Alu.mult,
                    accum_out=acc_t[:, col : col + 1],
                ).then_inc(sem_dve, 1)

            vector.wait_ge(sem_dve, 1)  # id_t engine-write visible
            vector.wait_ge(sem_pe, 1)
            diag("mainA", 0)
            # pts diffs (gathered x already weighted; y = weighted gt)
            vector.wait_ge(sem_pt, 16)
            vector.wait_ge(sem_gA, 16)
            nc.vector.tensor_sub(
                pd3[:, 0:4, :],
                gt3[:, 0:4, D : D + PC].bitcast(f8e4),
                pg3[:, 0:4, :],
            ).then_inc(sem_dve, 1)
            vector.wait_ge(sem_gB, 16)
            nc.vector.tensor_sub(
                pd3[:, 4:8, :],
                gt3[:, 4:8, D : D + PC].bitcast(f8e4),
                pg3[:, 4:8, :],
            ).then_inc(sem_dve, 1)
            vector.wait_ge(sem_pe, 2)
            diag("gg", 1)
            vector.wait_ge(sem_pe, 3)
            diag("mainB", 2)

        @block.scalar
        def _(scalar):
            scalar.wait_ge(sem_qv, 16)
            scalar.wait_ge(sem_cp, 16)
            nc.scalar.activation(
                lq_t[:], qy_t[:], Act.Ln, bias=cp_t[:, 32:33], scale=2.5
            ).then_inc(sem_act, 1)
            scalar.wait_ge(sem_dve, 2)  # bd diff done
            nc.scalar.activation(
                bd_t[:], bd_t[:], Act.Square, accum_out=acc_t[:, 5:6]
            ).then_inc(sem_act, 1)
            scalar.wait_ge(sem_dve, 4)  # pd_A diff done
            nc.scalar.activation(
                pd3[:, 0:4, :], pd3[:, 0:4, :], Act.Square,
                accum_out=acc_t[:, 3:4],
            ).then_inc(sem_act, 1)
            scalar.wait_ge(sem_dve, 5)  # pd_B diff done
            nc.scalar.activation(
                pd3[:, 4:8, :], pd3[:, 4:8, :], Act.Square,
                accum_out=acc_t[:, 4:5],
            ).then_inc(sem_act, 1)

    nc.compile()
    return nc


def _get_nc(vector_dims: int):
    key = ("nc", vector_dims)
    if key not in _CACHE:
        _CACHE[key] = _build_bass(vector_dims)
    return _CACHE[key]


def _pack_idx(idxs):
    """int idx array (n % 16 == 0) -> [128, n/32] f32 idx table
    (wrap-16, replicated to 128 partitions)."""
    idxs = np.asarray(idxs, dtype=np.int16)
    n = len(idxs)
    t = idxs.reshape(n // 16, 16).T  # [16, n/16]
    t = np.tile(t, (8, 1))  # [128, n/16]
    f = np.zeros((128, n // 32), dtype=np.float32)
    f.view(np.int16)[:] = t
    return f


def _prepare(inputs):
    import ml_dtypes

    e4 = ml_dtypes.float8_e4m3

    zs = np.asarray(inputs["zs"], dtype=np.float32)
    rzs = np.asarray(inputs["rzs"], dtype=np.float32)
    pts = np.asarray(inputs["pts"], dtype=np.float32)
    pts_gt = np.asarray(inputs["pts_gt"], dtype=np.float32)
    qy = np.asarray(inputs["qy"], dtype=np.float32)
    best = np.asarray(inputs["best"], dtype=np.float64)
    best_gt = np.asarray(inputs["best_gt"], dtype=np.float64)
    mapping = np.asarray(inputs["mapping"])
    vector_dims = int(np.asarray(inputs["vector_dims"]))

    w_p = np.ones(P, dtype=np.float64)
    w_p[list(MARKS)] += W_MARK
    w_sq = np.sqrt(w_p)
    wc = w_sq[None, None, :, None]

    zs_q = np.ascontiguousarray(zs.astype(e4))
    qv_q = (qy * np.float32(0.4 * vector_dims)).astype(e4)

    wpts_q = np.zeros((B, S, PCP), dtype=e4)
    wpts_q[:, :, :PC] = (pts * wc).astype(np.float32).astype(e4).reshape(B, S, PC)
    ptsgt_q = (pts_gt * wc).astype(np.float32).astype(e4).reshape(B, S, PC)

    gath_b = np.empty((B, S, K), dtype=np.uint8)
    gath_b[:, :, :D] = (np.float32(-2.0) * rzs).astype(e4).view(np.uint8)
    gath_b[:, :, D:] = wpts_q.view(np.uint8)

    best_w = (best * w_sq[None, :, None]).astype(np.float32)
    bestgt_w = (best_gt * w_sq[None, :, None]).astype(np.float32)

    in_maps = []
    for c in range(N_CORES):
        sl = slice(c * BL, (c + 1) * BL)
        map_c = mapping[sl].astype(np.int32)  # [BL, S]
        # gather token k = b_local*128 + s -> absolute row b_local*S + map
        idx_all = (
            np.arange(BL)[:, None] * S + map_c
        ).reshape(BL * S).astype(np.int16)
        cpk = np.zeros((S, NCONST), dtype=np.float32)
        cpk[:, 0:16] = _pack_idx(idx_all[0 : 4 * S])
        cpk[:, 16:32] = _pack_idx(idx_all[4 * S : 8 * S])
        cpk[:, 32] = np.float32(LN_B0)
        cpk[:P, 33 : 33 + BC] = best_w[sl].transpose(1, 0, 2).reshape(P, BC)
        cpk[:P, 49 : 49 + BC] = bestgt_w[sl].transpose(1, 0, 2).reshape(P, BC)
        in_maps.append(
            {
                "zs": zs_q[sl].reshape(BL * S, D),
                "gath": gath_b[sl].reshape(BL * S, K),
                "ptsgt": np.ascontiguousarray(
                    ptsgt_q[sl].transpose(1, 0, 2).reshape(S, BL * PC)
                ),
                "qv": np.ascontiguousarray(
                    qv_q[sl].transpose(1, 0, 2).reshape(S, BL * V)
                ),
                "cpack": cpk,
            }
        )
    return in_maps, vector_dims


def _combine(results) -> np.ndarray:
    total = np.float64(0.0)
    for r in results:
        por = r["po"].astype(np.float64)
        total += (
            por[:, 0:3].sum()
            + por[:, 3:5].sum() / (B * S * PC)
            + por[:, 5].sum() / (B * PC)
        )
    return np.float32(total)


def kernel(**inputs) -> np.ndarray:
    from concourse.bass_utils import run_bass_kernel_spmd

    in_maps, vector_dims = _prepare(inputs)
    nc = _get_nc(vector_dims)

    trace = os.environ.get("KERNEL_TRACE", "") == "1"
    res = run_bass_kernel_spmd(nc, in_maps, core_ids=list(range(N_CORES)), trace=trace)
    if trace and res.exec_time_ns is not None:
        print(f"HW exec time: {res.exec_time_ns} ns")

    return _combine(res.results)
